# revision 5
# baseline (speedup 1.0000x reference)
"""Trainium2 Bass kernel for 3x3 valid Conv2D (NCHW, OIHW), batch-parallel on 8 cores.

x(32,64,130,130) conv w(128,64,3,3) -> (32,128,128,128), plus bias(128,)
broadcast against the LAST axis (Wo) of the output (faithful to the
reference's torch-style broadcast, which requires Wo == K == 128).

Strategy per core (4 images):
  - x stored in SBUF with row-parity interleave: partitions 0-63 = channels
    (even image rows), partitions 64-127 = channels (odd image rows).
  - All matmuls are 64-deep (one tap = one matmul) issued as CONCURRENT
    pairs into disjoint PE row-groups: the even-rows PSUM tile's tap and the
    odd-rows tile's tap always live in opposite partition halves, so each
    ~216ns slot streams 2x64 rows = a full 128-wide PE. 9 slots per 8 output
    rows = 100% PE utilization with ZERO 128<->64 row-config transitions
    (the old pair+single scheme paid ~95ns twice per 16 rows).
  - The two 8-row tile-pairs of a 16-row block are interleaved slot-by-slot
    so consecutive accumulations into the SAME PSUM tile from different PE
    row-groups are >=1 slot (216ns) apart, clearing the ~173ns PSUM write
    pipeline drain (back-to-back same-tile row-group switches race the
    read-modify-write accumulation in flight and abort on hardware).
  - bf16 matmuls (~2.0e-3 rel err vs the fp32 reference; accumulation fp32).
  - x DMA'd in 8 uniform chunks per image (9 row-pairs, 1-row overlap) on
    the sync queue; weights split in two pieces on the scalar queue so the
    first tap block lands early. PE clock warmed during the DMA head via
    matmuls on a vector-memset tile.
  - DVE evicts PSUM -> row-interleaved SBUF staging [128, 2048]; one 1MB
    DMA per 16 rows (8KB contiguous per output channel) on the scalar queue.
  - Tail: the last image's final block evicts even rows on DVE and odd rows
    on the scalar (activation) engine in parallel, and stores the two 8-row
    halves on different DMA queues (sync is idle by then).
"""
import numpy as np

B, C, K, H, W = 32, 64, 128, 130, 130
HO = WO = 128
NCORES = 8
BLOC = B // NCORES  # 4 images per core
T = 65              # parity row-pairs (rows 0..129 -> 65 even + 65 odd)
NG = 16             # groups of 8 output rows per image
TC = 9              # row-pairs per x chunk (2 groups + 1 overlap row)
NCHUNK = 8
COMPUTE = "bf16"
TAIL_SPLIT = True    # last-block: parallel evict engines + per-sub stores on both queues
VEC_MEMSET = True    # warmup memset on vector instead of gpsimd
W_SPLIT = True       # weights DMA in two pieces

_CACHE = {}


def _build(with_bias: bool, compute: str = "bf16"):
    import concourse.tile as tile
    from concourse import bacc, mybir

    nc = bacc.Bacc("TRN2", target_bir_lowering=False, debug=False)
    f32 = mybir.dt.float32
    cdt = mybir.dt.float32r if compute == "f32r" else mybir.dt.bfloat16

    x_d = nc.dram_tensor("xloc", [BLOC, 128, T * W], cdt, kind="ExternalInput")
    w_d = nc.dram_tensor("wpk", [128, 1152], cdt, kind="ExternalInput")
    o_d = nc.dram_tensor("out", [BLOC, K, HO, WO], f32, kind="ExternalOutput")
    if with_bias:
        b_d = nc.dram_tensor("btile", [128, 512], f32, kind="ExternalInput")

    o_flat = o_d.ap().rearrange("b k i j -> b k (i j)")
    x_flat = x_d.ap().rearrange("b p (t j) -> b p t j", j=W)

    with tile.TileContext(nc) as tc:
        with (
            tc.tile_pool(name="wpool", bufs=1) as wpool,
            tc.tile_pool(name="xpool", bufs=12) as xpool,
            tc.tile_pool(name="spool", bufs=4) as spool,
            tc.tile_pool(name="psum", bufs=2, space="PSUM") as psum,
        ):
            # weights in two pieces so the first tap block lands early
            wt = wpool.tile([128, 1152], cdt)
            if W_SPLIT:
                nc.scalar.dma_start(wt[:, 0:384], w_d.ap()[:, 0:384])
                nc.scalar.dma_start(wt[:, 384:1152], w_d.ap()[:, 384:1152])
            else:
                nc.scalar.dma_start(wt[:], w_d.ap()[:, :])
            if with_bias:
                bt = wpool.tile([128, 512], f32, tag="bias")
                nc.scalar.dma_start(bt[:], b_d.ap()[:, :])

            # warm the PE clock gate during the DMA head; vector memset so
            # the warmup isn't gated on gpsimd's slower engine init
            warmsrc = wpool.tile([128, 512], cdt, tag="warmsrc")
            if VEC_MEMSET:
                nc.vector.memset(warmsrc[:], 0.0)
            else:
                nc.gpsimd.memset(warmsrc[:], 0.0)
            wpe = psum.tile([128, 512], f32, tag="pe")
            wpo = psum.tile([128, 512], f32, tag="po")
            for _ in range(4):
                nc.tensor.matmul(wpe[:], warmsrc[:, 0:128], warmsrc[:],
                                 start=True, stop=True)
                nc.tensor.matmul(wpo[:], warmsrc[:, 0:128], warmsrc[:],
                                 start=True, stop=True)

            ptags = [("pe", "po"), ("pe2", "po2")]
            for b in range(BLOC):
                xvs = []
                for c in range(NCHUNK):
                    xt = xpool.tile([128, TC * W], cdt)
                    nc.sync.dma_start(xt[:], x_flat[b, :, 8 * c:8 * c + TC, :])
                    xvs.append(xt[:].rearrange("p (t j) -> p t j", j=W))

                for hblk in range(NG // 2):
                    last_blk = TAIL_SPLIT and (b == BLOC - 1 and hblk == NG // 2 - 1)
                    st = spool.tile([128, 2048], f32)
                    sv = st[:].rearrange("p (r j) -> p r j", j=WO)
                    xv = xvs[hblk]
                    subs = [(xv, 0), (xv, 4)]
                    pe0 = psum.tile([128, 512], f32, tag="pe")
                    po0 = psum.tile([128, 512], f32, tag="po")
                    pe1 = psum.tile([128, 512], f32, tag="pe2")
                    po1 = psum.tile([128, 512], f32, tag="po2")
                    tiles = [(pe0, po0), (pe1, po1)]
                    # 9 slots of concurrent 64-deep tap pairs per tile-pair;
                    # the even-rows (pe) tap and odd-rows (po) tap always use
                    # opposite partition halves. Sub-interleaved so same-tile
                    # row-group switches are a full slot apart.
                    for u in range(3):
                        tEo = 1 if u == 2 else 0
                        tOo = 0 if u == 0 else 1
                        eL, eH = (0, 64) if u != 1 else (64, 128)
                        oL, oH = (64, 128) if u != 1 else (0, 64)
                        for vi in range(3):
                            col = 128 * (3 * u + vi)
                            first = (u == 0 and vi == 0)
                            final = (u == 2 and vi == 2)
                            for sub, (xv_, lm) in enumerate(subs):
                                pe, po = tiles[sub]
                                nc.tensor.matmul(
                                    pe[:], wt[eL:eH, col:col + 128],
                                    xv_[eL:eH, lm + tEo:lm + tEo + 4, vi:vi + 128],
                                    start=first, stop=final,
                                )
                                nc.tensor.matmul(
                                    po[:], wt[oL:oH, col:col + 128],
                                    xv_[oL:oH, lm + tOo:lm + tOo + 4, vi:vi + 128],
                                    start=first, stop=final,
                                )
                    for sub in range(2):
                        pe, po = tiles[sub]
                        ev = sv[:, 8 * sub:8 * sub + 8:2, :]
                        ov = sv[:, 8 * sub + 1:8 * sub + 8:2, :]
                        if with_bias:
                            nc.vector.tensor_add(ev, pe[:], bt[:])
                            nc.vector.tensor_add(ov, po[:], bt[:])
                        elif last_blk:
                            # parallel eviction across two engines
                            nc.vector.tensor_copy(ev, pe[:])
                            nc.scalar.copy(ov, po[:])
                        else:
                            nc.vector.tensor_copy(ev, pe[:])
                            nc.vector.tensor_copy(ov, po[:])
                        if last_blk:
                            # per-sub 8-row stores on different DMA queues
                            r0 = (16 * hblk + 8 * sub) * WO
                            s0 = 1024 * sub
                            eng = nc.sync if sub == 0 else nc.scalar
                            eng.dma_start(o_flat[b, :, r0:r0 + 8 * WO],
                                          st[:, s0:s0 + 1024])
                    if not last_blk:
                        nc.scalar.dma_start(
                            o_flat[b, :, (16 * hblk) * WO:(16 * hblk + 16) * WO],
                            st[:],
                        )
    nc.compile()
    return nc


def _get_nc(with_bias: bool, compute: str = None):
    compute = compute or COMPUTE
    key = ("conv", with_bias, compute)
    if key not in _CACHE:
        _CACHE[key] = _build(with_bias, compute)
    return _CACHE[key]


def _prep_inputs(x, weight, bias, with_bias, compute: str = None):
    compute = compute or COMPUTE
    xs = x.reshape(NCORES, BLOC, C, H, W)
    xr = np.empty((NCORES, BLOC, 128, T * W), np.float32)
    xr[:, :, 0:64] = xs[:, :, :, 0::2, :].reshape(NCORES, BLOC, C, T * W)
    xr[:, :, 64:128] = xs[:, :, :, 1::2, :].reshape(NCORES, BLOC, C, T * W)

    wkc = np.ascontiguousarray(weight.transpose(2, 3, 1, 0))  # [u, v, c, k]
    wpk = np.empty((128, 1152), np.float32)
    for u in range(3):
        for v in range(3):
            col = 128 * (3 * u + v)
            wpk[0:64, col:col + 128] = wkc[u, v]
            wpk[64:128, col:col + 128] = wkc[u, v]

    if compute == "bf16":
        import ml_dtypes
        xr = xr.astype(ml_dtypes.bfloat16)
        wpk = wpk.astype(ml_dtypes.bfloat16)
    in_maps = []
    for core in range(NCORES):
        m = {"xloc": xr[core], "wpk": wpk}
        if with_bias:
            m["btile"] = np.tile(bias, (128, 4))  # bias[j] along free dim
        in_maps.append(m)
    return in_maps


def kernel(x, weight, bias):
    from concourse.bass_utils import run_bass_kernel_spmd

    x = np.ascontiguousarray(np.asarray(x, dtype=np.float32))
    weight = np.asarray(weight, dtype=np.float32)
    bias = np.asarray(bias, dtype=np.float32)
    with_bias = bool(np.any(bias))

    nc = _get_nc(with_bias)
    in_maps = _prep_inputs(x, weight, bias, with_bias)
    res = run_bass_kernel_spmd(nc, in_maps, core_ids=list(range(NCORES)))
    out = np.empty((B, K, HO, WO), np.float32)
    for core in range(NCORES):
        out[core * BLOC:(core + 1) * BLOC] = res.results[core]["out"]
    return out


# revision 7
# speedup vs baseline: 1.1373x; 1.1373x over previous
"""Trainium2 Bass kernel for 3x3 valid Conv2D (NCHW, OIHW), batch-parallel on 8 cores.

x(32,64,130,130) conv w(128,64,3,3) -> (32,128,128,128), plus bias(128,)
broadcast against the LAST axis (Wo) of the output (faithful to the
reference's torch-style broadcast, which requires Wo == K == 128).

Strategy per core (4 images):
  - x stored in SBUF with row-parity interleave: partitions 0-63 = channels
    (even image rows), partitions 64-127 = channels (odd image rows). A tap
    pair (u, u+1) then reads both halves at ONE free-dim offset, so two
    64-deep taps fuse into one 128-deep matmul -- no data duplication.
  - Per 4 same-parity output rows (one PSUM tile [128k, 512px]):
    3 fused pair-matmuls (128-deep) + 3 single-tap matmuls (64-deep); the
    even-tile single (rows 0-63) and odd-tile single (rows 64-127) are
    issued back-to-back into distinct PE row-groups so they run
    concurrently. => 9 matmul slots per 8 output rows = 100% PE array
    utilization. (An all-64-deep variant with zero row-config transitions
    was tried and is ~19% SLOWER: sustained dual-row-group streaming drops
    the PE clock from ~2.37GHz to ~2.0GHz; short dual-64 bursts are fine.)
  - Pairs/singles order ALTERNATES per 16-row block, so the 128<->64 PE
    row-config transition (~95ns) is paid once per block, not twice.
  - Sub-major ordering (each 8-row tile-pair finishes all its matmuls
    before the next starts): the very first matmuls only wait on the first
    5-row-pair x chunk, and the final block's first tile-pair evicts
    ~1.3us before the last matmul, shrinking head and tail.
  - bf16 matmuls (~2.0e-3 rel err vs the fp32 reference; accumulation fp32).
  - x DMA'd in chunks (9 row-pairs, 1-row overlap) on the sync queue;
    weights in two pieces on the scalar queue so the pair-tap block lands
    early. PE clock warmed during the DMA head via matmuls on a
    vector-memset tile (vector's engine init ends ~0.5us before gpsimd's).
  - DVE evicts PSUM -> row-interleaved SBUF staging [128, 2048]; one 1MB
    DMA per 16 rows (8KB contiguous per output channel) on the scalar queue.
  - Tail: the last block evicts even rows on DVE and odd rows on the scalar
    (activation) engine in parallel and stores each 8-row tile-pair on the
    sync queue (idle by then) as soon as it is staged.
"""
import numpy as np

B, C, K, H, W = 32, 64, 128, 130, 130
HO = WO = 128
NCORES = 8
BLOC = B // NCORES  # 4 images per core
T = 65              # parity row-pairs (rows 0..129 -> 65 even + 65 odd)
NG = 16             # groups of 8 output rows per image
TC = 9              # row-pairs per x chunk (2 groups + 1 overlap row)
NCHUNK = 8
COMPUTE = "bf16"
TAIL_SPLIT = True    # last-block: parallel evict engines + stores on sync queue
VEC_MEMSET = True    # warmup memset on vector instead of gpsimd
W_SPLIT = True       # weights DMA in two pieces
WARM_N, WARM_W = 8, 448  # warmup matmul count / moving width

_CACHE = {}


def _build(with_bias: bool, compute: str = "bf16"):
    import concourse.tile as tile
    from concourse import bacc, mybir

    nc = bacc.Bacc("TRN2", target_bir_lowering=False, debug=False)
    f32 = mybir.dt.float32
    cdt = mybir.dt.float32r if compute == "f32r" else mybir.dt.bfloat16

    x_d = nc.dram_tensor("xloc", [BLOC, 128, T * W], cdt, kind="ExternalInput")
    w_d = nc.dram_tensor("wpk", [128, 1152], cdt, kind="ExternalInput")
    o_d = nc.dram_tensor("out", [BLOC, K, HO, WO], f32, kind="ExternalOutput")
    if with_bias:
        b_d = nc.dram_tensor("btile", [128, 512], f32, kind="ExternalInput")

    o_flat = o_d.ap().rearrange("b k i j -> b k (i j)")
    x_flat = x_d.ap().rearrange("b p (t j) -> b p t j", j=W)

    with tile.TileContext(nc) as tc:
        with (
            tc.tile_pool(name="wpool", bufs=1) as wpool,
            tc.tile_pool(name="xpool", bufs=12) as xpool,
            tc.tile_pool(name="xpool0", bufs=1) as xpool0,
            tc.tile_pool(name="spool", bufs=4) as spool,
            tc.tile_pool(name="psum", bufs=2, space="PSUM") as psum,
        ):
            # weights: pair taps (cols 0:768) first so the opening pair
            # matmuls aren't gated on the single-tap block
            wt = wpool.tile([128, 1152], cdt)
            if W_SPLIT:
                nc.scalar.dma_start(wt[:, 0:768], w_d.ap()[:, 0:768])
                nc.scalar.dma_start(wt[:, 768:1152], w_d.ap()[:, 768:1152])
            else:
                nc.scalar.dma_start(wt[:], w_d.ap()[:, :])
            if with_bias:
                bt = wpool.tile([128, 512], f32, tag="bias")
                nc.scalar.dma_start(bt[:], b_d.ap()[:, :])

            # warm the PE clock gate during the DMA head
            warmsrc = wpool.tile([128, 512], cdt, tag="warmsrc")
            if VEC_MEMSET:
                nc.vector.memset(warmsrc[:], 0.0)
            else:
                nc.gpsimd.memset(warmsrc[:], 0.0)
            wpe = psum.tile([128, 512], f32, tag="pe")
            wpo = psum.tile([128, 512], f32, tag="po")
            for _ in range(WARM_N // 2):
                nc.tensor.matmul(wpe[:, 0:WARM_W], warmsrc[:, 0:128],
                                 warmsrc[:, 0:WARM_W], start=True, stop=True)
                nc.tensor.matmul(wpo[:, 0:WARM_W], warmsrc[:, 0:128],
                                 warmsrc[:, 0:WARM_W], start=True, stop=True)

            for b in range(BLOC):
                xvs = []
                for c in range(NCHUNK):
                    if b == 0 and c == 0:
                        # split the very first chunk so the opening tile-pair
                        # only waits on 5 row-pairs of input
                        xa = xpool0.tile([128, 5 * W], cdt, tag="xa")
                        nc.sync.dma_start(xa[:], x_flat[0, :, 0:5, :])
                        xb = xpool0.tile([128, 5 * W], cdt, tag="xb")
                        nc.sync.dma_start(xb[:], x_flat[0, :, 4:9, :])
                        xvs.append(None)
                        continue
                    xt = xpool.tile([128, TC * W], cdt)
                    nc.sync.dma_start(xt[:], x_flat[b, :, 8 * c:8 * c + TC, :])
                    xvs.append(xt[:].rearrange("p (t j) -> p t j", j=W))

                for hblk in range(NG // 2):
                    last_blk = TAIL_SPLIT and (b == BLOC - 1 and hblk == NG // 2 - 1)
                    st = spool.tile([128, 2048], f32)
                    sv = st[:].rearrange("p (r j) -> p r j", j=WO)
                    xv = xvs[hblk]
                    if xv is None:
                        xv0 = xa[:].rearrange("p (t j) -> p t j", j=W)
                        xv1 = xb[:].rearrange("p (t j) -> p t j", j=W)
                        subs = [(xv0, 0), (xv1, 0)]
                    else:
                        subs = [(xv, 0), (xv, 4)]
                    pe0 = psum.tile([128, 512], f32, tag="pe")
                    po0 = psum.tile([128, 512], f32, tag="po")
                    pe1 = psum.tile([128, 512], f32, tag="pe2")
                    po1 = psum.tile([128, 512], f32, tag="po2")
                    tiles = [(pe0, po0), (pe1, po1)]
                    # alternating order: one PE row-config transition per
                    # 16-row block instead of two
                    sing_first = (hblk % 2 == 1)

                    def emit_pairs(sub, first, final):
                        xv_, lm = subs[sub]
                        pe, po = tiles[sub]
                        for vi in range(3):
                            nc.tensor.matmul(
                                pe[:], wt[:, 128 * vi:128 * (vi + 1)],
                                xv_[:, lm:lm + 4, vi:vi + 128],
                                start=(first and vi == 0),
                                stop=(final and vi == 2),
                            )
                            nc.tensor.matmul(
                                po[:], wt[:, 384 + 128 * vi:384 + 128 * (vi + 1)],
                                xv_[:, lm + 1:lm + 5, vi:vi + 128],
                                start=(first and vi == 0),
                                stop=(final and vi == 2),
                            )

                    def emit_singles(sub, first, final):
                        xv_, lm = subs[sub]
                        pe, po = tiles[sub]
                        for vi in range(3):
                            nc.tensor.matmul(
                                pe[:], wt[0:64, 768 + 128 * vi:768 + 128 * (vi + 1)],
                                xv_[0:64, lm + 1:lm + 5, vi:vi + 128],
                                start=(first and vi == 0),
                                stop=(final and vi == 2),
                            )
                            nc.tensor.matmul(
                                po[:], wt[64:128, 768 + 128 * vi:768 + 128 * (vi + 1)],
                                xv_[64:128, lm:lm + 4, vi:vi + 128],
                                start=(first and vi == 0),
                                stop=(final and vi == 2),
                            )

                    def evict(sub):
                        pe, po = tiles[sub]
                        ev = sv[:, 8 * sub:8 * sub + 8:2, :]
                        ov = sv[:, 8 * sub + 1:8 * sub + 8:2, :]
                        if with_bias:
                            nc.vector.tensor_add(ev, pe[:], bt[:])
                            nc.vector.tensor_add(ov, po[:], bt[:])
                        elif last_blk:
                            # parallel eviction across two engines
                            nc.vector.tensor_copy(ev, pe[:])
                            nc.scalar.copy(ov, po[:])
                        else:
                            nc.vector.tensor_copy(ev, pe[:])
                            nc.vector.tensor_copy(ov, po[:])
                        if last_blk:
                            # store each 8-row tile-pair immediately on the
                            # (idle) sync queue
                            r0 = (16 * hblk + 8 * sub) * WO
                            s0 = 1024 * sub
                            nc.sync.dma_start(o_flat[b, :, r0:r0 + 8 * WO],
                                              st[:, s0:s0 + 1024])

                    if sing_first:
                        emit_singles(0, True, False)
                        emit_singles(1, True, False)
                        emit_pairs(0, False, True)
                        evict(0)
                        emit_pairs(1, False, True)
                        evict(1)
                    else:
                        emit_pairs(0, True, False)
                        emit_pairs(1, True, False)
                        emit_singles(0, False, True)
                        evict(0)
                        emit_singles(1, False, True)
                        evict(1)
                    if not last_blk:
                        nc.scalar.dma_start(
                            o_flat[b, :, (16 * hblk) * WO:(16 * hblk + 16) * WO],
                            st[:],
                        )
    nc.compile()
    return nc


def _get_nc(with_bias: bool, compute: str = None):
    compute = compute or COMPUTE
    key = ("conv", with_bias, compute)
    if key not in _CACHE:
        _CACHE[key] = _build(with_bias, compute)
    return _CACHE[key]


def _prep_inputs(x, weight, bias, with_bias, compute: str = None):
    compute = compute or COMPUTE
    xs = x.reshape(NCORES, BLOC, C, H, W)
    xr = np.empty((NCORES, BLOC, 128, T * W), np.float32)
    xr[:, :, 0:64] = xs[:, :, :, 0::2, :].reshape(NCORES, BLOC, C, T * W)
    xr[:, :, 64:128] = xs[:, :, :, 1::2, :].reshape(NCORES, BLOC, C, T * W)

    wkc = np.ascontiguousarray(weight.transpose(2, 3, 1, 0))  # [u, v, c, k]
    wpk = np.empty((128, 1152), np.float32)
    for v in range(3):
        wpk[0:64, 128 * v:128 * (v + 1)] = wkc[0, v]        # even pair lower: u0
        wpk[64:128, 128 * v:128 * (v + 1)] = wkc[1, v]      # even pair upper: u1
        wpk[0:64, 384 + 128 * v:384 + 128 * (v + 1)] = wkc[1, v]    # odd pair lower: u1
        wpk[64:128, 384 + 128 * v:384 + 128 * (v + 1)] = wkc[2, v]  # odd pair upper: u2
        wpk[0:64, 768 + 128 * v:768 + 128 * (v + 1)] = wkc[2, v]    # even single: u2
        wpk[64:128, 768 + 128 * v:768 + 128 * (v + 1)] = wkc[0, v]  # odd single: u0

    if compute == "bf16":
        import ml_dtypes
        xr = xr.astype(ml_dtypes.bfloat16)
        wpk = wpk.astype(ml_dtypes.bfloat16)
    in_maps = []
    for core in range(NCORES):
        m = {"xloc": xr[core], "wpk": wpk}
        if with_bias:
            m["btile"] = np.tile(bias, (128, 4))  # bias[j] along free dim
        in_maps.append(m)
    return in_maps


def kernel(x, weight, bias):
    from concourse.bass_utils import run_bass_kernel_spmd

    x = np.ascontiguousarray(np.asarray(x, dtype=np.float32))
    weight = np.asarray(weight, dtype=np.float32)
    bias = np.asarray(bias, dtype=np.float32)
    with_bias = bool(np.any(bias))

    nc = _get_nc(with_bias)
    in_maps = _prep_inputs(x, weight, bias, with_bias)
    res = run_bass_kernel_spmd(nc, in_maps, core_ids=list(range(NCORES)))
    out = np.empty((B, K, HO, WO), np.float32)
    for core in range(NCORES):
        out[core * BLOC:(core + 1) * BLOC] = res.results[core]["out"]
    return out
